# revision 28
# baseline (speedup 1.0000x reference)
"""RGCN (basis-decomposition, one-hot features) message passing on 8 trn2 NeuronCores.

Math (per reference):
    weight[r] = sum_b comp[r,b] * basis[b]          # [R, N, OUT]
    msg_e     = weight[edge_type_e, src_e]          # [E, OUT]
    agg       = segment_sum(msg, dst) / max(cnt, 1) # mean aggregation
    out       = log_softmax(relu(agg + root + bias), axis=1)

Distribution strategy: shard edges by DESTINATION node range across the 8
cores (each core owns N/8 output rows), so no collectives are needed.

Per 128-edge block the device:
  1. gathers, per edge, the 5 basis rows of the src node from an fp8e4
     table (basisQ[n] = 32 * basis[:, n, :], 1536 B contiguous per node)
     via gpsimd dma_gather,
  2. accumulates psum[tile] += lhsT_b.T @ basisrows_b for b = 0..4, where
     lhsT_b[e, nd] = comp[type_e, b] * (dst_e == nd) is a host-prepared
     comp-scaled one-hot block (pure scatter of comp values driven by the
     edge structure; no arithmetic), streamed sequentially from DRAM in
     fp8e4.  This keeps DVE free: no on-device one-hot construction.
The epilogue (mean incl. the 1/32 dequant, +root, +bias, relu,
log_softmax) runs fused per tile on DVE/ACT and streams the rows out.

Host-side work is limited to index metadata (sharding, grouping edges by
destination tile, padding, bincount, scatter of comp values into one-hot
positions) and data layout (transpose/pad/quantize of input tensors).
All floating-point arithmetic on tensor data runs on device.
"""

import math
from contextlib import ExitStack
from dataclasses import dataclass

import numpy as np

import ml_dtypes

import concourse.bacc as bacc
import concourse.bass as bass
import concourse.mybir as mybir
import concourse.tile as tile
from concourse.bass_utils import run_bass_kernel_spmd

F32 = mybir.dt.float32
F8 = mybir.dt.float8e4
I16 = mybir.dt.int16
NP_F8 = ml_dtypes.float8_e4m3
P = 128  # SBUF partitions


@dataclass
class Cfg:
    n_nodes: int = 50000
    n_rel: int = 5
    n_bases: int = 5
    out_dim: int = 300
    n_cores: int = 8
    g_blk: int = 8          # gather-group size in 128-edge blocks (<=8: 1024 idx/call)
    scale: float = 32.0     # power-of-2 exponent shift for the fp8 basis table
    double_row: bool = True   # fp8 DoubleRow matmuls (2 edge blocks / matmul)
    swdge_queues: int = 4   # SWDGE queues to round-robin dma_gather over
    act_preload: bool = True  # preload the Exp+Ln act table set once
    balance: bool = False   # degree-balanced dst relabeling across tiles

    @property
    def rowp(self) -> int:  # elements per fp8 basisQ row (256B multiple)
        return (self.row + 255) // 256 * 256

    @property
    def half(self) -> int:  # src-range half size for int16 gather indices
        return (self.n_nodes + 1) // 2

    @property
    def row(self) -> int:  # payload elements per basisQ row
        return self.n_bases * self.out_dim

    @property
    def lrow(self) -> int:  # elements per lhsT slot row (5 one-hot rows)
        return self.n_rel * P

    @property
    def npc(self) -> int:  # nodes per core, multiple of 128
        return ((self.n_nodes + self.n_cores - 1) // self.n_cores + P - 1) // P * P

    @property
    def n_tiles(self) -> int:
        return self.npc // P


# ----------------------------------------------------------------------------
# Host-side prep: index metadata + layout only (no float math on tensor data)
# ----------------------------------------------------------------------------

def host_prep(cfg: Cfg, edge_index, edge_type, basis, root, bias, comp):
    src = np.asarray(edge_index[0]).astype(np.int64)
    dst = np.asarray(edge_index[1]).astype(np.int64)
    etype = np.asarray(edge_type).astype(np.int64)
    basis = np.asarray(basis, dtype=np.float32)
    root = np.asarray(root, dtype=np.float32)
    bias = np.asarray(bias, dtype=np.float32)
    comp = np.asarray(comp, dtype=np.float32)

    N, R, B, OUT = cfg.n_nodes, cfg.n_rel, cfg.n_bases, cfg.out_dim
    NPC, T = cfg.npc, cfg.n_tiles
    HALF = cfg.half

    # fp8 gather table: basisQ[n] = scale * basis[:, n, :] (exponent shift)
    basisQ = np.zeros((N, cfg.rowp), dtype=NP_F8)
    basisQ[:, : B * OUT] = (
        basis.transpose(1, 0, 2).reshape(N, B * OUT) * cfg.scale
    ).astype(NP_F8)
    comp_f8 = comp.astype(NP_F8)  # [R, B] values copied into one-hot slots
    biast = np.ascontiguousarray(np.broadcast_to(bias[None, :], (P, OUT)))

    # Degree-balanced relabeling of dst nodes: nodes sorted by in-degree and
    # snake-dealt across all (core, tile) bins so every 128-node tile carries
    # a near-equal edge count.  perm[n] = new id; undone on the host when the
    # sharded outputs are stitched (pure metadata).
    if cfg.balance:
        deg = np.bincount(dst, minlength=N)
        n_tiles_all = cfg.n_cores * T
        order = np.argsort(-deg, kind="stable")
        perm = np.empty(N, dtype=np.int64)
        fill = np.zeros(n_tiles_all, dtype=np.int64)
        fwd = np.arange(n_tiles_all)
        snake = np.concatenate([fwd, fwd[::-1]])
        pos_in_snake = 0
        for i, n in enumerate(order):
            while True:
                tb = snake[pos_in_snake % snake.size]
                pos_in_snake += 1
                if fill[tb] < P:
                    break
            perm[n] = tb * P + fill[tb]
            fill[tb] += 1
        dst = perm[dst]
    else:
        perm = None

    core_of = dst // NPC
    per_core = []
    counts = np.zeros((cfg.n_cores, T, 2), dtype=np.int64)
    for c in range(cfg.n_cores):
        m = core_of == c
        s_c, t_c, dl_c = src[m], etype[m], dst[m] - c * NPC
        tid = dl_c // P
        hid = s_c // HALF
        order = np.argsort(tid * 2 + hid, kind="stable")
        s_c, t_c, dl_c = s_c[order], t_c[order], dl_c[order]
        tid, hid = tid[order], hid[order]
        for t in range(T):
            for h in range(2):
                counts[c, t, h] = np.count_nonzero((tid == t) & (hid == h))
        per_core.append((s_c, t_c, dl_c))

    # cells: (tile, half, n_blocks, max_real) — shared static structure
    # across cores; max_real caps the gather row count below n_blocks*128
    cells = []
    for t in range(T):
        tile_cells = []
        for h in range(2):
            mx = int(counts[:, t, h].max())
            Bc = int(math.ceil(mx / P))
            if Bc > 0:
                tile_cells.append((t, h, Bc, mx))
        if not tile_cells:
            tile_cells.append((t, 0, 1, 16))  # empty tile still zeroes psum
        cells.extend(tile_cells)
    NB = sum(c[2] for c in cells)

    in_maps = []
    for c in range(cfg.n_cores):
        s_c, t_c, dl_c = per_core[c]
        srcs = np.zeros((NB * P,), dtype=np.int64)
        typs = np.zeros((NB * P,), dtype=np.int64)
        dloc = np.zeros((NB * P,), dtype=np.int64)
        valid = np.zeros((NB * P,), dtype=bool)
        pos = np.concatenate([[0], np.cumsum(counts[c].reshape(-1))])
        off = 0
        for (t, h, Bc, _) in cells:
            gi = t * 2 + h
            a, b = pos[gi], pos[gi + 1]
            n = b - a
            srcs[off : off + n] = s_c[a:b] - h * HALF
            typs[off : off + n] = t_c[a:b]
            dloc[off : off + n] = dl_c[a:b] - t * P
            valid[off : off + n] = True
            off += Bc * P

        # comp-scaled one-hot lhsT blocks, partition-major for a clean stream:
        # lhsT[p, j, b, nd] = comp[type, b] iff slot (j*128+p) carries an edge
        # with local dst nd.  Pure scatter of comp values (no arithmetic).
        lhsT = np.zeros((P, NB, R, P), dtype=NP_F8)
        sl = np.arange(NB * P)
        pv, jv = sl[valid] % P, sl[valid] // P
        lhsT[pv, jv, :, dloc[valid]] = comp_f8[typs[valid], :]

        # int16 gather indices wrapped in 16 partitions, replicated to 128
        idx16 = np.ascontiguousarray(
            np.tile(srcs.astype(np.int16).reshape(NB * 8, 16).T, (8, 1))
        )  # [128, NB*8] int16
        idx_cols = idx16.view(np.float32)  # [128, NB*4]

        cnt = np.zeros((NPC,), dtype=np.float32)
        np.add.at(cnt, dl_c, 1.0)
        cnt2 = cnt.reshape(T, P).T.copy()

        rootp = np.zeros((NPC, OUT), dtype=np.float32)
        if perm is not None:
            inv = np.full(NPC * cfg.n_cores, -1, dtype=np.int64)
            inv[perm] = np.arange(N)
            sel = inv[c * NPC : (c + 1) * NPC]
            rootp[sel >= 0] = root[sel[sel >= 0]]
        else:
            lo, hi = c * NPC, min((c + 1) * NPC, N)
            if hi > lo:
                rootp[: hi - lo] = root[lo:hi]

        consts = np.concatenate([idx_cols, cnt2, biast], axis=1).copy()
        in_maps.append(dict(basisQ=basisQ, consts=consts, lhsT=lhsT, rootp=rootp))
    return cells, in_maps, bool(np.any(bias != 0.0)), perm


# ----------------------------------------------------------------------------
# Device program
# ----------------------------------------------------------------------------

def build_program(cfg: Cfg, cells, use_bias: bool):
    N, R, OUT, ROW = cfg.n_nodes, cfg.n_rel, cfg.out_dim, cfg.rowp
    NPC, T = cfg.npc, cfg.n_tiles
    HALF = cfg.half
    LROW = cfg.lrow
    NB = sum(c[2] for c in cells)
    IDXW = NB * 4  # f32 cols holding int16 gather indices

    nc = bacc.Bacc(
        "TRN2",
        target_bir_lowering=False,
        debug=False,
        enable_asserts=False,
        num_devices=cfg.n_cores,
        num_swdge_queues=cfg.swdge_queues,
    )
    W = IDXW + T + OUT  # packed const width
    basisQ = nc.dram_tensor("basisQ", [N, ROW], F8, kind="ExternalInput").ap()
    lhsT = nc.dram_tensor("lhsT", [P, NB, LROW], F8, kind="ExternalInput").ap()
    consts = nc.dram_tensor("consts", [P, W], F32, kind="ExternalInput").ap()
    rootp = nc.dram_tensor("rootp", [NPC, OUT], F32, kind="ExternalInput").ap()
    out = nc.dram_tensor("out", [NPC, OUT], F32, kind="ExternalOutput").ap()

    add = mybir.AluOpType.add

    with tile.TileContext(nc) as tc, ExitStack() as ctx:
        cpool = ctx.enter_context(tc.tile_pool(name="const", bufs=1))
        gpool = ctx.enter_context(tc.tile_pool(name="g", bufs=7))
        lpool = ctx.enter_context(tc.tile_pool(name="lhs", bufs=6))
        ppool = ctx.enter_context(tc.tile_pool(name="ps", bufs=4, space="PSUM"))
        rpool = ctx.enter_context(tc.tile_pool(name="root", bufs=3))
        epool = ctx.enter_context(tc.tile_pool(name="epi", bufs=3))

        # pre-zero the gather pool buffers once (first thing on DVE): rows
        # beyond a capped gather are never written, and 0 * anything keeps
        # psum exact; stale bytes from earlier groups are finite fp8.
        for _ in range(7):
            gz = gpool.tile([P, cfg.g_blk, ROW], F8, tag="g")
            nc.vector.memset(gz[:], 0.0)

        if cfg.act_preload:
            # One load of the table set covering BOTH Exp and Ln; without it
            # the act-table insertion pass alternates exp/ln sets per tile
            # (98 x 1283 ns of ACT_TABLE_LOAD).
            from concourse.hw_specs import get_activation_tables

            A = mybir.ActivationFunctionType
            tabs = list(get_activation_tables(nc.m.arch).values())
            set_id = next(
                i for i, s in enumerate(tabs) if A.Exp in s and A.Ln in s
            )
            pre = mybir.InstLoadActFuncSet(
                name=nc.get_next_instruction_name(),
                ins=[], outs=[], act_func_set_id=set_id,
            )
            pre.engine = mybir.EngineType.Activation
            nc.add_instruction(pre)

        consts_sb = cpool.tile([P, W], F32)
        # idx area first (in chunks) so gather group 0 can start ASAP
        c0 = min(8 * cfg.g_blk, IDXW)
        nc.sync.dma_start(consts_sb[:, 0:c0], consts[:, 0:c0])
        nc.sync.dma_start(consts_sb[:, c0:IDXW], consts[:, c0:IDXW])
        nc.sync.dma_start(consts_sb[:, IDXW:W], consts[:, IDXW:W])
        idx_area = consts_sb[:, 0:IDXW]
        cnt_sb = consts_sb[:, IDXW : IDXW + T]
        biast_sb = consts_sb[:, IDXW + T : IDXW + T + OUT]
        # mean reciprocal folded with the fp8 table dequant (1/scale)
        rcnt_sb = cpool.tile([P, T], F32)
        nc.vector.tensor_scalar_max(rcnt_sb[:], cnt_sb, 1.0)
        nc.vector.reciprocal(rcnt_sb[:], rcnt_sb[:])
        nc.vector.tensor_scalar(
            out=rcnt_sb[:], in0=rcnt_sb[:], scalar1=1.0 / cfg.scale,
            scalar2=None, op0=mybir.AluOpType.mult,
        )

        # group cells by tile, preserving stream order
        by_tile = [[] for _ in range(T)]
        jstart = 0
        for (t, h, Bc, mx) in cells:
            by_tile[t].append((h, Bc, jstart, mx))
            jstart += Bc

        gq = 0
        lq = 0
        for t in range(T):
            tile_cells = by_tile[t]
            n_blocks = sum(Bc for (_, Bc, _, _) in tile_cells)
            psum_t = ppool.tile([P, OUT], F32)
            root_t = rpool.tile([P, OUT], F32)
            nc.sync.dma_start(root_t[:], rootp[t * P : (t + 1) * P, :])
            # total matmul count for this tile (start/stop bookkeeping)
            if cfg.double_row:
                n_mm = 0
                for (_, Bc, _, _) in tile_cells:
                    for g0 in range(0, Bc, cfg.g_blk):
                        nb = min(cfg.g_blk, Bc - g0)
                        n_mm += (nb // 2 + nb % 2) * R
            else:
                n_mm = n_blocks * R
            done = 0
            for (h, Bc, jbase, mx) in tile_cells:
                for g0 in range(0, Bc, cfg.g_blk):
                    nb = min(cfg.g_blk, Bc - g0)
                    gb = jbase + g0
                    # cap the gather to the max real row count across cores
                    nidx = min(nb * P, (mx - g0 * P + 15) // 16 * 16)
                    gt = gpool.tile([P, cfg.g_blk, ROW], F8, tag="g")
                    hi = min((h + 1) * HALF, N)
                    nc.gpsimd.dma_gather(
                        out_ap=gt[:, :nb, :],
                        in_ap=basisQ[h * HALF : hi, :],
                        idxs_ap=idx_area[:, gb * 4 : (gb + nb) * 4].bitcast(I16),
                        num_idxs=nidx,
                        num_idxs_reg=nidx,
                        elem_size=ROW,
                        queue_num=gq,
                    )
                    gq = (gq + 1) % cfg.swdge_queues
                    lt = lpool.tile([P, cfg.g_blk, R, P], F8, tag="l")
                    leng = (nc.scalar, nc.sync)[lq % 2]
                    lq += 1
                    leng.dma_start(lt[:, :nb, :, :], lhsT[:, gb : gb + nb, :])
                    if cfg.double_row:
                        j = 0
                        while j < nb:
                            if j + 1 < nb:
                                for b in range(R):
                                    nc.tensor.matmul(
                                        psum_t[:],
                                        lhsT=lt[:, j : j + 2, b, :],
                                        rhs=gt[:, j : j + 2, b * OUT : (b + 1) * OUT],
                                        start=(done == 0),
                                        stop=(done == n_mm - 1),
                                        perf_mode=mybir.MatmulPerfMode.DoubleRow,
                                    )
                                    done += 1
                                j += 2
                            else:
                                for b in range(R):
                                    nc.tensor.matmul(
                                        psum_t[:],
                                        lhsT=lt[:, j, b, :],
                                        rhs=gt[:, j, b * OUT : (b + 1) * OUT],
                                        start=(done == 0),
                                        stop=(done == n_mm - 1),
                                    )
                                    done += 1
                                j += 1
                    else:
                        for j in range(nb):
                            for b in range(R):
                                nc.tensor.matmul(
                                    psum_t[:],
                                    lhsT=lt[:, j, b, :],
                                    rhs=gt[:, j, b * OUT : (b + 1) * OUT],
                                    start=(done == 0),
                                    stop=(done == n_mm - 1),
                                )
                                done += 1

            # epilogue: mean(+dequant) on ACT, +root(+bias), relu, log_softmax
            h_t = epool.tile([P, OUT], F32, tag="h")
            nc.scalar.activation(
                out=h_t[:], in_=psum_t[:],
                func=mybir.ActivationFunctionType.Copy,
                scale=rcnt_sb[:, t : t + 1],
            )
            nc.vector.tensor_tensor(out=h_t[:], in0=h_t[:], in1=root_t[:], op=add)
            if use_bias:
                nc.vector.tensor_tensor(
                    out=h_t[:], in0=h_t[:], in1=biast_sb[:], op=add
                )
            nc.vector.tensor_scalar_max(h_t[:], h_t[:], 0.0)
            mx = epool.tile([P, 2], F32, tag="mx")
            nc.vector.tensor_reduce(
                out=mx[:, 0:1], in_=h_t[:], axis=mybir.AxisListType.X,
                op=mybir.AluOpType.max, negate=True,
            )
            ex = epool.tile([P, OUT], F32, tag="ex")
            nc.scalar.activation(
                out=ex[:], in_=h_t[:], func=mybir.ActivationFunctionType.Exp,
                bias=mx[:, 0:1], scale=1.0, accum_out=mx[:, 1:2],
            )
            ln = epool.tile([P, 2], F32, tag="ln")
            nc.scalar.activation(
                out=ln[:, 0:1], in_=mx[:, 1:2], func=mybir.ActivationFunctionType.Ln,
            )
            tot = epool.tile([P, 2], F32, tag="tot")
            nc.vector.tensor_tensor(
                out=tot[:, 0:1], in0=mx[:, 0:1], in1=ln[:, 0:1],
                op=mybir.AluOpType.subtract,
            )
            o = epool.tile([P, OUT], F32, tag="o")
            # per-partition bias add on ACT: DVE tensor_scalar with an AP
            # scalar measures ~5us/op vs ~0.5us here
            nc.scalar.activation(
                out=o[:], in_=h_t[:],
                func=mybir.ActivationFunctionType.Identity,
                bias=tot[:, 0:1], scale=1.0,
            )
            nc.sync.dma_start(out[t * P : (t + 1) * P, :], o[:])
    nc.compile()
    return nc


# ----------------------------------------------------------------------------
# Entry point
# ----------------------------------------------------------------------------

def _run(cfg: Cfg, inputs: dict, trace: bool = False):
    cells, in_maps, use_bias, perm = host_prep(
        cfg,
        inputs["edge_index"], inputs["edge_type"], inputs["basis"],
        inputs["root"], inputs["bias"], inputs["comp"],
    )
    nc = build_program(cfg, cells, use_bias)
    res = run_bass_kernel_spmd(
        nc, in_maps, core_ids=list(range(cfg.n_cores)), trace=trace,
    )
    parts = [res.results[c]["out"] for c in range(cfg.n_cores)]
    full = np.concatenate(parts, axis=0)
    if perm is not None:
        full = full[perm]  # undo the balanced relabeling
    else:
        full = full[: cfg.n_nodes]
    return np.ascontiguousarray(full.astype(np.float32)), res


def kernel(**inputs) -> np.ndarray:
    cfg = Cfg()
    out, _ = _run(cfg, inputs)
    return out


# revision 30
# speedup vs baseline: 1.0281x; 1.0281x over previous
"""RGCN (basis-decomposition, one-hot features) message passing on 8 trn2 NeuronCores.

Math (per reference):
    weight[r] = sum_b comp[r,b] * basis[b]          # [R, N, OUT]
    msg_e     = weight[edge_type_e, src_e]          # [E, OUT]
    agg       = segment_sum(msg, dst) / max(cnt, 1) # mean aggregation
    out       = log_softmax(relu(agg + root + bias), axis=1)

Distribution strategy: shard edges by DESTINATION node range across the 8
cores (each core owns N/8 output rows), so no collectives are needed.

Per 128-edge block the device:
  1. gathers, per edge, the 5 basis rows of the src node from an fp8e4
     table (basisQ[n] = 32 * basis[:, n, :], 1536 B contiguous per node)
     via gpsimd dma_gather,
  2. accumulates psum[tile] += lhsT_b.T @ basisrows_b for b = 0..4, where
     lhsT_b[e, nd] = comp[type_e, b] * (dst_e == nd) is a host-prepared
     comp-scaled one-hot block (pure scatter of comp values driven by the
     edge structure; no arithmetic), streamed sequentially from DRAM in
     fp8e4.  This keeps DVE free: no on-device one-hot construction.
The epilogue (mean incl. the 1/32 dequant, +root, +bias, relu,
log_softmax) runs fused per tile on DVE/ACT and streams the rows out.

Host-side work is limited to index metadata (sharding, grouping edges by
destination tile, padding, bincount, scatter of comp values into one-hot
positions) and data layout (transpose/pad/quantize of input tensors).
All floating-point arithmetic on tensor data runs on device.
"""

import math
from contextlib import ExitStack
from dataclasses import dataclass

import numpy as np

import ml_dtypes

import concourse.bacc as bacc
import concourse.bass as bass
import concourse.mybir as mybir
import concourse.tile as tile
from concourse.bass_utils import run_bass_kernel_spmd

F32 = mybir.dt.float32
F8 = mybir.dt.float8e4
I16 = mybir.dt.int16
NP_F8 = ml_dtypes.float8_e4m3
P = 128  # SBUF partitions


@dataclass
class Cfg:
    n_nodes: int = 50000
    n_rel: int = 5
    n_bases: int = 5
    out_dim: int = 300
    n_cores: int = 8
    g_blk: int = 8          # gather-group size in 128-edge blocks (<=8: 1024 idx/call)
    scale: float = 32.0     # power-of-2 exponent shift for the fp8 basis table
    double_row: bool = True   # fp8 DoubleRow matmuls (2 edge blocks / matmul)
    swdge_queues: int = 4   # SWDGE queues to round-robin dma_gather over
    act_preload: bool = True  # preload the Exp+Ln act table set once
    balance: bool = False   # degree-balanced dst relabeling across tiles

    @property
    def rowp(self) -> int:  # elements per fp8 basisQ row (256B multiple)
        return (self.row + 255) // 256 * 256

    @property
    def half(self) -> int:  # src-range half size for int16 gather indices
        return (self.n_nodes + 1) // 2

    @property
    def row(self) -> int:  # payload elements per basisQ row
        return self.n_bases * self.out_dim

    @property
    def lrow(self) -> int:  # elements per lhsT slot row (5 one-hot rows)
        return self.n_rel * P

    @property
    def npc(self) -> int:  # nodes per core, multiple of 128
        return ((self.n_nodes + self.n_cores - 1) // self.n_cores + P - 1) // P * P

    @property
    def n_tiles(self) -> int:
        return self.npc // P


# ----------------------------------------------------------------------------
# Host-side prep: index metadata + layout only (no float math on tensor data)
# ----------------------------------------------------------------------------

def host_prep(cfg: Cfg, edge_index, edge_type, basis, root, bias, comp):
    src = np.asarray(edge_index[0]).astype(np.int64)
    dst = np.asarray(edge_index[1]).astype(np.int64)
    etype = np.asarray(edge_type).astype(np.int64)
    basis = np.asarray(basis, dtype=np.float32)
    root = np.asarray(root, dtype=np.float32)
    bias = np.asarray(bias, dtype=np.float32)
    comp = np.asarray(comp, dtype=np.float32)

    N, R, B, OUT = cfg.n_nodes, cfg.n_rel, cfg.n_bases, cfg.out_dim
    NPC, T = cfg.npc, cfg.n_tiles
    HALF = cfg.half

    # fp8 gather table: basisQ[n] = scale * basis[:, n, :] (exponent shift)
    basisQ = np.zeros((N, cfg.rowp), dtype=NP_F8)
    basisQ[:, : B * OUT] = (
        basis.transpose(1, 0, 2).reshape(N, B * OUT) * cfg.scale
    ).astype(NP_F8)
    comp_f8 = comp.astype(NP_F8)  # [R, B] values copied into one-hot slots
    biast = np.ascontiguousarray(np.broadcast_to(bias[None, :], (P, OUT)))

    # Degree-balanced relabeling of dst nodes: nodes sorted by in-degree and
    # snake-dealt across all (core, tile) bins so every 128-node tile carries
    # a near-equal edge count.  perm[n] = new id; undone on the host when the
    # sharded outputs are stitched (pure metadata).
    if cfg.balance:
        deg = np.bincount(dst, minlength=N)
        n_tiles_all = cfg.n_cores * T
        order = np.argsort(-deg, kind="stable")
        perm = np.empty(N, dtype=np.int64)
        fill = np.zeros(n_tiles_all, dtype=np.int64)
        fwd = np.arange(n_tiles_all)
        snake = np.concatenate([fwd, fwd[::-1]])
        pos_in_snake = 0
        for i, n in enumerate(order):
            while True:
                tb = snake[pos_in_snake % snake.size]
                pos_in_snake += 1
                if fill[tb] < P:
                    break
            perm[n] = tb * P + fill[tb]
            fill[tb] += 1
        dst = perm[dst]
    else:
        perm = None

    core_of = dst // NPC
    per_core = []
    counts = np.zeros((cfg.n_cores, T, 2), dtype=np.int64)
    for c in range(cfg.n_cores):
        m = core_of == c
        s_c, t_c, dl_c = src[m], etype[m], dst[m] - c * NPC
        tid = dl_c // P
        hid = s_c // HALF
        order = np.argsort(tid * 2 + hid, kind="stable")
        s_c, t_c, dl_c = s_c[order], t_c[order], dl_c[order]
        tid, hid = tid[order], hid[order]
        for t in range(T):
            for h in range(2):
                counts[c, t, h] = np.count_nonzero((tid == t) & (hid == h))
        per_core.append((s_c, t_c, dl_c))

    # cells: (tile, half, n_blocks, max_real) — shared static structure
    # across cores; max_real caps the gather row count below n_blocks*128
    cells = []
    for t in range(T):
        tile_cells = []
        for h in range(2):
            mx = int(counts[:, t, h].max())
            Bc = int(math.ceil(mx / P))
            if Bc > 0:
                tile_cells.append((t, h, Bc, mx))
        if not tile_cells:
            tile_cells.append((t, 0, 1, 16))  # empty tile still zeroes psum
        cells.extend(tile_cells)
    NB = sum(c[2] for c in cells)

    in_maps = []
    for c in range(cfg.n_cores):
        s_c, t_c, dl_c = per_core[c]
        srcs = np.zeros((NB * P,), dtype=np.int64)
        typs = np.zeros((NB * P,), dtype=np.int64)
        dloc = np.zeros((NB * P,), dtype=np.int64)
        valid = np.zeros((NB * P,), dtype=bool)
        pos = np.concatenate([[0], np.cumsum(counts[c].reshape(-1))])
        off = 0
        for (t, h, Bc, _) in cells:
            gi = t * 2 + h
            a, b = pos[gi], pos[gi + 1]
            n = b - a
            srcs[off : off + n] = s_c[a:b] - h * HALF
            typs[off : off + n] = t_c[a:b]
            dloc[off : off + n] = dl_c[a:b] - t * P
            valid[off : off + n] = True
            off += Bc * P

        # comp-scaled one-hot lhsT blocks, partition-major for a clean stream:
        # lhsT[p, j, b, nd] = comp[type, b] iff slot (j*128+p) carries an edge
        # with local dst nd.  Pure scatter of comp values (no arithmetic).
        lhsT = np.zeros((P, NB, R, P), dtype=NP_F8)
        sl = np.arange(NB * P)
        pv, jv = sl[valid] % P, sl[valid] // P
        lhsT[pv, jv, :, dloc[valid]] = comp_f8[typs[valid], :]

        # int16 gather indices wrapped in 16 partitions, replicated to 128
        idx16 = np.ascontiguousarray(
            np.tile(srcs.astype(np.int16).reshape(NB * 8, 16).T, (8, 1))
        )  # [128, NB*8] int16
        idx_cols = idx16.view(np.float32)  # [128, NB*4]

        cnt = np.zeros((NPC,), dtype=np.float32)
        np.add.at(cnt, dl_c, 1.0)
        cnt2 = cnt.reshape(T, P).T.copy()

        rootp = np.zeros((NPC, OUT), dtype=np.float32)
        if perm is not None:
            inv = np.full(NPC * cfg.n_cores, -1, dtype=np.int64)
            inv[perm] = np.arange(N)
            sel = inv[c * NPC : (c + 1) * NPC]
            rootp[sel >= 0] = root[sel[sel >= 0]]
        else:
            lo, hi = c * NPC, min((c + 1) * NPC, N)
            if hi > lo:
                rootp[: hi - lo] = root[lo:hi]

        consts = np.concatenate([idx_cols, cnt2, biast], axis=1).copy()
        in_maps.append(dict(basisQ=basisQ, consts=consts, lhsT=lhsT, rootp=rootp))
    return cells, in_maps, bool(np.any(bias != 0.0)), perm


# ----------------------------------------------------------------------------
# Device program
# ----------------------------------------------------------------------------

def build_program(cfg: Cfg, cells, use_bias: bool):
    N, R, OUT, ROW = cfg.n_nodes, cfg.n_rel, cfg.out_dim, cfg.rowp
    NPC, T = cfg.npc, cfg.n_tiles
    HALF = cfg.half
    LROW = cfg.lrow
    NB = sum(c[2] for c in cells)
    IDXW = NB * 4  # f32 cols holding int16 gather indices

    nc = bacc.Bacc(
        "TRN2",
        target_bir_lowering=False,
        debug=False,
        enable_asserts=False,
        num_devices=cfg.n_cores,
        num_swdge_queues=cfg.swdge_queues,
    )
    W = IDXW + T + OUT  # packed const width
    basisQ = nc.dram_tensor("basisQ", [N, ROW], F8, kind="ExternalInput").ap()
    lhsT = nc.dram_tensor("lhsT", [P, NB, LROW], F8, kind="ExternalInput").ap()
    consts = nc.dram_tensor("consts", [P, W], F32, kind="ExternalInput").ap()
    rootp = nc.dram_tensor("rootp", [NPC, OUT], F32, kind="ExternalInput").ap()
    out = nc.dram_tensor("out", [NPC, OUT], F32, kind="ExternalOutput").ap()

    add = mybir.AluOpType.add

    with tile.TileContext(nc) as tc, ExitStack() as ctx:
        cpool = ctx.enter_context(tc.tile_pool(name="const", bufs=1))
        gpool = ctx.enter_context(tc.tile_pool(name="g", bufs=5))
        lpool = ctx.enter_context(tc.tile_pool(name="lhs", bufs=4))
        ppool = ctx.enter_context(tc.tile_pool(name="ps", bufs=4, space="PSUM"))
        rpool = ctx.enter_context(tc.tile_pool(name="root", bufs=3))
        epool = ctx.enter_context(tc.tile_pool(name="epi", bufs=3))

        # pre-zero the first two blocks of each gather pool buffer (first
        # thing on DVE): only tail calls (nb <= 2) can leave stale rows that
        # a matmul later reads, and those stale rows sit in blocks 0-1; all
        # full-group reads are fully written.  0 * anything keeps psum exact
        # and stale bytes from earlier groups are finite fp8.
        for _ in range(5):
            gz = gpool.tile([P, cfg.g_blk, ROW], F8, tag="g")
            nc.vector.memset(gz[:, 0:2, :], 0.0)

        if cfg.act_preload:
            # One load of the table set covering BOTH Exp and Ln; without it
            # the act-table insertion pass alternates exp/ln sets per tile
            # (98 x 1283 ns of ACT_TABLE_LOAD).
            from concourse.hw_specs import get_activation_tables

            A = mybir.ActivationFunctionType
            tabs = list(get_activation_tables(nc.m.arch).values())
            set_id = next(
                i for i, s in enumerate(tabs) if A.Exp in s and A.Ln in s
            )
            pre = mybir.InstLoadActFuncSet(
                name=nc.get_next_instruction_name(),
                ins=[], outs=[], act_func_set_id=set_id,
            )
            pre.engine = mybir.EngineType.Activation
            nc.add_instruction(pre)

        consts_sb = cpool.tile([P, W], F32)
        # idx area first (in chunks) so gather group 0 can start ASAP
        c0 = min(8 * cfg.g_blk, IDXW)
        nc.sync.dma_start(consts_sb[:, 0:c0], consts[:, 0:c0])
        nc.sync.dma_start(consts_sb[:, c0:IDXW], consts[:, c0:IDXW])
        nc.sync.dma_start(consts_sb[:, IDXW:W], consts[:, IDXW:W])
        idx_area = consts_sb[:, 0:IDXW]
        cnt_sb = consts_sb[:, IDXW : IDXW + T]
        biast_sb = consts_sb[:, IDXW + T : IDXW + T + OUT]
        # mean reciprocal folded with the fp8 table dequant (1/scale)
        rcnt_sb = cpool.tile([P, T], F32)
        nc.vector.tensor_scalar_max(rcnt_sb[:], cnt_sb, 1.0)
        nc.vector.reciprocal(rcnt_sb[:], rcnt_sb[:])
        nc.vector.tensor_scalar(
            out=rcnt_sb[:], in0=rcnt_sb[:], scalar1=1.0 / cfg.scale,
            scalar2=None, op0=mybir.AluOpType.mult,
        )

        # group cells by tile, preserving stream order
        by_tile = [[] for _ in range(T)]
        jstart = 0
        for (t, h, Bc, mx) in cells:
            by_tile[t].append((h, Bc, jstart, mx))
            jstart += Bc

        gq = 0
        lq = 0
        for t in range(T):
            tile_cells = by_tile[t]
            n_blocks = sum(Bc for (_, Bc, _, _) in tile_cells)
            psum_t = ppool.tile([P, OUT], F32)
            root_t = rpool.tile([P, OUT], F32)
            nc.sync.dma_start(root_t[:], rootp[t * P : (t + 1) * P, :])
            # total matmul count for this tile (start/stop bookkeeping)
            if cfg.double_row:
                n_mm = 0
                for (_, Bc, _, _) in tile_cells:
                    for g0 in range(0, Bc, cfg.g_blk):
                        nb = min(cfg.g_blk, Bc - g0)
                        n_mm += (nb // 2 + nb % 2) * R
            else:
                n_mm = n_blocks * R
            done = 0
            for (h, Bc, jbase, mx) in tile_cells:
                for g0 in range(0, Bc, cfg.g_blk):
                    nb = min(cfg.g_blk, Bc - g0)
                    gb = jbase + g0
                    # cap the gather to the max real row count across cores
                    nidx = min(nb * P, (mx - g0 * P + 15) // 16 * 16)
                    gt = gpool.tile([P, cfg.g_blk, ROW], F8, tag="g")
                    hi = min((h + 1) * HALF, N)
                    nc.gpsimd.dma_gather(
                        out_ap=gt[:, :nb, :],
                        in_ap=basisQ[h * HALF : hi, :],
                        idxs_ap=idx_area[:, gb * 4 : (gb + nb) * 4].bitcast(I16),
                        num_idxs=nidx,
                        num_idxs_reg=nidx,
                        elem_size=ROW,
                        queue_num=gq,
                    )
                    gq = (gq + 1) % cfg.swdge_queues
                    lt = lpool.tile([P, cfg.g_blk, R, P], F8, tag="l")
                    leng = (nc.scalar, nc.sync)[lq % 2]
                    lq += 1
                    leng.dma_start(lt[:, :nb, :, :], lhsT[:, gb : gb + nb, :])
                    if cfg.double_row:
                        j = 0
                        while j < nb:
                            if j + 1 < nb:
                                for b in range(R):
                                    nc.tensor.matmul(
                                        psum_t[:],
                                        lhsT=lt[:, j : j + 2, b, :],
                                        rhs=gt[:, j : j + 2, b * OUT : (b + 1) * OUT],
                                        start=(done == 0),
                                        stop=(done == n_mm - 1),
                                        perf_mode=mybir.MatmulPerfMode.DoubleRow,
                                    )
                                    done += 1
                                j += 2
                            else:
                                for b in range(R):
                                    nc.tensor.matmul(
                                        psum_t[:],
                                        lhsT=lt[:, j, b, :],
                                        rhs=gt[:, j, b * OUT : (b + 1) * OUT],
                                        start=(done == 0),
                                        stop=(done == n_mm - 1),
                                    )
                                    done += 1
                                j += 1
                    else:
                        for j in range(nb):
                            for b in range(R):
                                nc.tensor.matmul(
                                    psum_t[:],
                                    lhsT=lt[:, j, b, :],
                                    rhs=gt[:, j, b * OUT : (b + 1) * OUT],
                                    start=(done == 0),
                                    stop=(done == n_mm - 1),
                                )
                                done += 1

            # epilogue: mean(+dequant) on ACT, +root(+bias), relu, log_softmax
            h_t = epool.tile([P, OUT], F32, tag="h")
            nc.scalar.activation(
                out=h_t[:], in_=psum_t[:],
                func=mybir.ActivationFunctionType.Copy,
                scale=rcnt_sb[:, t : t + 1],
            )
            nc.vector.tensor_tensor(out=h_t[:], in0=h_t[:], in1=root_t[:], op=add)
            if use_bias:
                nc.vector.tensor_tensor(
                    out=h_t[:], in0=h_t[:], in1=biast_sb[:], op=add
                )
            nc.vector.tensor_scalar_max(h_t[:], h_t[:], 0.0)
            mx = epool.tile([P, 2], F32, tag="mx")
            nc.vector.tensor_reduce(
                out=mx[:, 0:1], in_=h_t[:], axis=mybir.AxisListType.X,
                op=mybir.AluOpType.max, negate=True,
            )
            ex = epool.tile([P, OUT], F32, tag="ex")
            nc.scalar.activation(
                out=ex[:], in_=h_t[:], func=mybir.ActivationFunctionType.Exp,
                bias=mx[:, 0:1], scale=1.0, accum_out=mx[:, 1:2],
            )
            ln = epool.tile([P, 2], F32, tag="ln")
            nc.scalar.activation(
                out=ln[:, 0:1], in_=mx[:, 1:2], func=mybir.ActivationFunctionType.Ln,
            )
            tot = epool.tile([P, 2], F32, tag="tot")
            nc.vector.tensor_tensor(
                out=tot[:, 0:1], in0=mx[:, 0:1], in1=ln[:, 0:1],
                op=mybir.AluOpType.subtract,
            )
            o = epool.tile([P, OUT], F32, tag="o")
            # per-partition bias add on ACT: DVE tensor_scalar with an AP
            # scalar measures ~5us/op vs ~0.5us here
            nc.scalar.activation(
                out=o[:], in_=h_t[:],
                func=mybir.ActivationFunctionType.Identity,
                bias=tot[:, 0:1], scale=1.0,
            )
            nc.sync.dma_start(out[t * P : (t + 1) * P, :], o[:])
    nc.compile()
    return nc


# ----------------------------------------------------------------------------
# Entry point
# ----------------------------------------------------------------------------

def _run(cfg: Cfg, inputs: dict, trace: bool = False):
    cells, in_maps, use_bias, perm = host_prep(
        cfg,
        inputs["edge_index"], inputs["edge_type"], inputs["basis"],
        inputs["root"], inputs["bias"], inputs["comp"],
    )
    nc = build_program(cfg, cells, use_bias)
    res = run_bass_kernel_spmd(
        nc, in_maps, core_ids=list(range(cfg.n_cores)), trace=trace,
    )
    parts = [res.results[c]["out"] for c in range(cfg.n_cores)]
    full = np.concatenate(parts, axis=0)
    if perm is not None:
        full = full[perm]  # undo the balanced relabeling
    else:
        full = full[: cfg.n_nodes]
    return np.ascontiguousarray(full.astype(np.float32)), res


def kernel(**inputs) -> np.ndarray:
    cfg = Cfg()
    out, _ = _run(cfg, inputs)
    return out
